# revision 11
# baseline (speedup 1.0000x reference)
"""Multi-head attention (QKV proj + RoPE + SDPA + out proj) on 8 TRN2 NeuronCores.

Sharding: batch x head-group. Core c handles batch c//4 and heads
4*(c%4) .. 4*(c%4)+3 (4 of 16 heads, 256 of 1024 feature dims).

Per-core kernel. All matmul operands are bf16 (fp32 PSUM accumulation):
bf16 streams 1 row/cycle at the warm 2.4 GHz PE clock and enables the
compiler's fast-weight-load path (disabled for fp32), so LDWEIGHTS hides
behind the matmul stream. Inputs arrive from HBM already in bf16 (half
the DMA bytes of the fp32 original).

  - QKV projections from host-transposed xT [1024, 2048]:
      Q,K feature-major [dims, tokens] (for scores contraction over head_dim)
      V token-major [tokens, dims] (for attn@V contraction over keys)
  - RoPE on Q/K in feature-major form: q_rot = F0*q + F1*(Pswap@q), with
    F0/F1 [128, S] precomputed on host from freqs_cis and Pswap a constant
    pair-swap permutation matmul.
  - scores computed TRANSPOSED: s[k, q] = sum_d K[d,k] Q[d,q]  (so that
    attn@V needs no transpose); exp via ACT directly from 2-bank PSUM
    groups [128, 1024] with the 1/sqrt(64) scale folded in; no max
    subtraction (|scores/8| <~ 12, safe in f32).
  - attn@V with a ones-row appended per head (M=65): row 64 accumulates
    the softmax denominator for free.
  - normalize straight out of PSUM: ACT reciprocal of the denominator row
    + rank-1 ones x recip broadcast matmul + one DVE multiply into bf16.
  - output projection row-parallel: each core emits a [2048, 1024] bf16
    partial; host sums partials per batch and adds wo_b + wo_w @ wv_b
    (the V bias commutes through softmax-weighted sum: sum_k attn = 1).

Host gather: out[b] = sum_{hg} partial[4*b+hg] + wo_b + wo_w @ wv_b.
"""

import numpy as np
import ml_dtypes

import concourse.bass as bass
import concourse.mybir as mybir
import concourse.tile as tile
from concourse import bacc
import concourse.bass_utils as _bu
from concourse.bass_utils import run_bass_kernel_spmd

F32 = mybir.dt.float32
F32R = mybir.dt.float32r
F16 = mybir.dt.float16
BF16 = mybir.dt.bfloat16
I16 = mybir.dt.int16
PEXP_A = 23.083120654223414   # 128*log2(e)/8
PEXP_B = 16250.8875           # 128*127 - 5.1125 (truncation-tuned magic)
AF = mybir.ActivationFunctionType
OP = mybir.AluOpType

B, S, D = 2, 2048, 1024
NH, HD = 16, 64
NCORES = 8
HPC = 4          # heads per core
DL = HPC * HD    # 256 local dims per core

# set by test harness to request an NTFF trace
TRACE = False
USE_WARM = True
LAST_RESULTS = [None]


def _build_module():
    nc = bacc.Bacc("TRN2", target_bir_lowering=False, debug=False)

    xt_d = nc.dram_tensor("xt", [D, S], F16, kind="ExternalInput")
    wqt_d = nc.dram_tensor("wqt", [D, DL], F16, kind="ExternalInput")
    wkt_d = nc.dram_tensor("wkt", [D, DL], F16, kind="ExternalInput")
    wvt_d = nc.dram_tensor("wvt", [D, DL], F16, kind="ExternalInput")
    wot_d = nc.dram_tensor("wot", [DL, D], F16, kind="ExternalInput")
    qb_d = nc.dram_tensor("qb2", [128, 2], F32, kind="ExternalInput")
    kb_d = nc.dram_tensor("kb2", [128, 2], F32, kind="ExternalInput")
    f0_d = nc.dram_tensor("f0", [128, S], F16, kind="ExternalInput")
    f1_d = nc.dram_tensor("f1", [128, S], F16, kind="ExternalInput")
    psw_d = nc.dram_tensor("pswap", [128, 128], F16, kind="ExternalInput")
    o4_d = nc.dram_tensor("ones4", [128, 4], BF16, kind="ExternalInput")
    out_d = nc.dram_tensor("partial", [S, D], F16, kind="ExternalOutput")

    with tile.TileContext(nc) as tc:
        with (
            tc.tile_pool(name="wts", bufs=1) as wpool,
            tc.tile_pool(name="persist", bufs=1) as ppool,
        ):
            # ---- weights / constants (resident) ----
            wqt = wpool.tile([128, 8, DL], F16, tag="wqt")
            nc.sync.dma_start(
                out=wqt[:], in_=wqt_d.ap().rearrange("(dc p) m -> p dc m", p=128))
            xt_re0 = xt_d.ap().rearrange("(dc p) t -> p dc t", p=128)
            xt0_sb = wpool.tile([128, 8, 512], F16, tag="xt0")
            for dc in range(8):
                nc.sync.dma_start(out=xt0_sb[:, dc, :],
                                  in_=xt_re0[:, dc, 0:512])

            wkt = wpool.tile([128, 8, DL], F16, tag="wkt")
            nc.sync.dma_start(
                out=wkt[:], in_=wkt_d.ap().rearrange("(dc p) m -> p dc m", p=128))
            wvt = wpool.tile([128, 8, DL], F16, tag="wvt")
            nc.sync.dma_start(
                out=wvt[:], in_=wvt_d.ap().rearrange("(dc p) m -> p dc m", p=128))
            qb = wpool.tile([128, 2], F32, tag="qb")
            nc.sync.dma_start(out=qb[:], in_=qb_d.ap())
            kb = wpool.tile([128, 2], F32, tag="kb")
            nc.sync.dma_start(out=kb[:], in_=kb_d.ap())
            f0 = wpool.tile([128, S], F16, tag="f0")
            nc.sync.dma_start(out=f0[:], in_=f0_d.ap())
            f1 = wpool.tile([128, S], F16, tag="f1")
            nc.sync.dma_start(out=f1[:], in_=f1_d.ap())
            psw = wpool.tile([128, 128], F16, tag="pswap")
            nc.sync.dma_start(out=psw[:], in_=psw_d.ap())
            o4 = wpool.tile([128, 4], BF16, tag="o4")
            nc.sync.dma_start(out=o4[:], in_=o4_d.ap())
            wot = wpool.tile([128, 2, D], F16, tag="wot")
            nc.sync.dma_start(
                out=wot[:], in_=wot_d.ap().rearrange("(pt p) o -> p pt o", p=128))

            # ---- persistent activations ----
            qrot = [ppool.tile([128, S], F16, tag=f"qrot{pt}", name=f"qrot{pt}") for pt in range(2)]
            krot = [ppool.tile([128, S], F16, tag=f"krot{pt}", name=f"krot{pt}") for pt in range(2)]
            ynorm = [ppool.tile([128, S], F16, tag=f"ynorm{pt}", name=f"ynorm{pt}") for pt in range(2)]
            vsb = [ppool.tile([128, 260], BF16, tag=f"v{kt}", name=f"vsb{kt}") for kt in range(16)]

            # preload the ACT exp table set and the GpSimd broadcast
            # library during the DMA lead-in (the first partition_broadcast
            # otherwise pays a ~7us library load at a block tail)
            warmact = wpool.tile([1, 1], F32, tag="warmact")
            nc.vector.memset(warmact[:], 0.0)
            nc.scalar.activation(warmact[:], warmact[:], AF.Exp, scale=1.0)
            gwi = wpool.tile([1, 8], F32, tag="gwi")
            nc.vector.memset(gwi[:], 1.0)
            gwo = wpool.tile([8, 8], F32, tag="gwo")
            nc.gpsimd.partition_broadcast(gwo[:], gwi[:], channels=8)

            xt_re = xt_d.ap().rearrange("(dc p) t -> p dc t", p=128)

            # ---- phase 1: QKV projections + RoPE ----
            with (
                tc.tile_pool(name="xt", bufs=2) as xpool,
                tc.tile_pool(name="ptmp", bufs=3) as tpool,
                tc.tile_pool(name="ps2", bufs=2, space="PSUM") as ps2,
            ):
                for qc in range(4):
                    tsl = slice(qc * 512, (qc + 1) * 512)
                    if qc == 0:
                        xt_sb = xt0_sb
                    else:
                        xt_sb = xpool.tile([128, 8, 512], F16, tag="xt")
                        nc.sync.dma_start(out=xt_sb[:], in_=xt_re[:, :, tsl])

                    for wt, bvec, rot in ((wqt, qb, qrot), (wkt, kb, krot)):
                        for pt in range(2):
                            qp = ps2.tile([128, 512], F32, tag="proj")
                            for dc in range(8):
                                nc.tensor.matmul(
                                    qp[:],
                                    wt[:, dc, pt * 128:(pt + 1) * 128],
                                    xt_sb[:, dc, :],
                                    start=(dc == 0), stop=(dc == 7))
                            qsb = tpool.tile([128, 512], F16, tag="qsb")
                            nc.scalar.activation(
                                qsb[:], qp[:], AF.Identity,
                                bias=bvec[:, pt:pt + 1], scale=1.0)
                            sw = ps2.tile([128, 512], F32, tag="swap")
                            nc.tensor.matmul(
                                sw[:], psw[:], qsb[:], start=True, stop=True)
                            t0 = tpool.tile([128, 512], F16, tag="t0")
                            nc.vector.tensor_tensor(
                                t0[:], qsb[:], f0[:, tsl], OP.mult)
                            t1 = tpool.tile([128, 512], F16, tag="t1")
                            nc.vector.tensor_tensor(
                                t1[:], sw[:], f1[:, tsl], OP.mult)
                            nc.vector.tensor_tensor(
                                rot[pt][:, tsl], t0[:], t1[:], OP.add)

                    for tt in range(4):
                        kt = qc * 4 + tt
                        vp = ps2.tile([128, 256], F32, tag="vps")
                        for dc in range(8):
                            nc.tensor.matmul(
                                vp[:],
                                xt_sb[:, dc, tt * 128:(tt + 1) * 128],
                                wvt[:, dc, :],
                                start=(dc == 0), stop=(dc == 7))
                        # single strided ACT copy: vp [128,(4h 64d)] ->
                        # vsb [128, 4h x 65] (leaves the ones column alone)
                        nc.scalar.activation(
                            vsb[kt][:, 0:260].rearrange(
                                "p (h x) -> p h x", x=65)[:, :, 0:64],
                            vp[:].rearrange("p (h d) -> p h d", d=64),
                            AF.Copy)
                        nc.vector.tensor_copy(vsb[kt][:, 64:260:65], o4[:])

            # ---- phase 2: attention ----
            # Heads in pairs: the two 64-row score matmuls use disjoint PE
            # row groups (base partitions 0/64) and run concurrently.
            # "Warm" matmuls are dependency-free full-array fillers written
            # into psum that is about to be overwritten anyway (start=True
            # clears it); they keep the HAM activity monitor at K=8/8
            # through pipeline-fill bubbles.
            #
            # The out-projection for qcp=0 is WOVEN into the (qcp=1, hp=0)
            # attention block: its matmuls fill the PE gaps of the ACT
            # (exp)-paced score/attnV pipeline, and its PSUM->SBUF drains go
            # to the otherwise-idle DVE. Only qcp=1's out-projection runs
            # serially at the end (with ACT/DVE alternating drains).
            with (
                tc.tile_pool(name="exp", bufs=6) as epool,
                tc.tile_pool(name="ysb", bufs=2) as ypool,
                tc.tile_pool(name="opool", bufs=4) as opool,
                tc.tile_pool(name="ps3s", bufs=1, space="PSUM") as ps3s,
                tc.tile_pool(name="ps3y", bufs=1, space="PSUM") as ps3y,
            ):
                def warm_run(n, name):
                    if not USE_WARM:
                        return
                    wt_ = ps3s.tile([128, 512], F32, tag="s0", name=name)
                    for _ in range(n):
                        nc.tensor.matmul(wt_[:], psw[:], wot[:, 0, 0:512],
                                         start=True, stop=True)

                def emit_outproj_group(j, tt, oc, drain):
                    op = ps3s.tile([128, 512], F32, tag=f"s{j % 2}",
                                   name=f"op{tt}_{oc}")
                    for pt2 in range(2):
                        nc.tensor.matmul(
                            op[:],
                            ynorm[pt2][:, tt * 128:(tt + 1) * 128],
                            wot[:, pt2, oc * 512:(oc + 1) * 512],
                            start=(pt2 == 0), stop=(pt2 == 1))
                    osb = opool.tile([128, 512], F16, tag="osb",
                                     name=f"osb{tt}_{oc}")
                    if drain == "dve":
                        nc.vector.tensor_copy(osb[:], op[:])
                    else:
                        nc.scalar.activation(osb[:], op[:], AF.Copy)
                    nc.sync.dma_start(
                        out=out_d.ap()[tt * 128:(tt + 1) * 128,
                                       oc * 512:(oc + 1) * 512],
                        in_=osb[:])

                for qcp in range(2):
                    q0 = qcp * 1024
                    for hp in range(2):
                        pt = hp
                        if (qcp, hp) != (0, 0):
                            warm_run(4, f"warmhp{qcp}_{hp}")
                        yps = [ps3y.tile([65, 1024], F32, tag=f"y{i}",
                                         name=f"yp{i}") for i in range(2)]
                        # out-proj groups of the previous qcp to weave into
                        # this block's PE gaps (2 per kt over kt=2..9)
                        woven = ([(j, t, o) for j, (t, o) in enumerate(
                                     (t, o) for t in range(8) for o in range(2))]
                                 if (qcp == 1 and hp == 0) else [])

                        def emit_scores_exp(kt, nwarm):
                            exs = []
                            for i in range(2):
                                sp = ps3s.tile([128, 1024], F32, tag=f"s{i}",
                                               name=f"sp{i}")
                                for w in range(nwarm if USE_WARM else 0):
                                    nc.tensor.matmul(
                                        sp[:, 0:512], psw[:],
                                        wot[:, 0, 0:512],
                                        start=True, stop=True)
                                po = 64 * i
                                for qh in range(2):
                                    nc.tensor.matmul(
                                        sp[:, qh * 512:(qh + 1) * 512],
                                        krot[pt][po:po + 64,
                                                 kt * 128:(kt + 1) * 128],
                                        qrot[pt][po:po + 64,
                                                 q0 + qh * 512:q0 + (qh + 1) * 512],
                                        start=True, stop=True)
                                ex = epool.tile([128, 1024], BF16, tag="e",
                                                name=f"ex{i}")
                                if i == 1:
                                    # offload this quarter of the softmax to
                                    # the idle DVE via the exp bit trick
                                    # (~3.3% max weight err, averages out)
                                    nc.scalar.activation(
                                        ex[:, 0:512], sp[:, 0:512], AF.Exp,
                                        scale=0.125)
                                    nc.vector.tensor_scalar(
                                        out=ex[:, 512:1024].bitcast(I16),
                                        in0=sp[:, 512:1024],
                                        scalar1=PEXP_A, scalar2=PEXP_B,
                                        op0=OP.mult, op1=OP.add)
                                else:
                                    nc.scalar.activation(ex[:], sp[:], AF.Exp,
                                                         scale=0.125)
                                exs.append(ex)
                            return exs

                        def emit_attnv(kt, exs):
                            for i in range(2):
                                h = 2 * hp + i
                                for qh in range(2):
                                    nc.tensor.matmul(
                                        yps[i][:, qh * 512:(qh + 1) * 512],
                                        vsb[kt][:, 65 * h:65 * h + 65],
                                        exs[i][:, qh * 512:(qh + 1) * 512],
                                        start=(kt == 0), stop=(kt == 15))

                        # software pipeline: attnV trails scores/exp by two
                        # iterations so exp never queues behind attnV and the
                        # previous block's normalization tail (which gates
                        # attnv(0) via the yps slot) stays off the PE path
                        exq = [emit_scores_exp(0, 2), emit_scores_exp(1, 1)]
                        for kt in range(2, 16):
                            exq.append(emit_scores_exp(kt, 1 if kt < 3 else 0))
                            emit_attnv(kt - 2, exq[kt - 2])
                            if woven and 2 <= kt <= 9:
                                for j, t, o in woven[(kt - 2) * 2:(kt - 1) * 2]:
                                    emit_outproj_group(j, t, o, "dve")
                        emit_attnv(14, exq[14])
                        emit_attnv(15, exq[15])

                        # normalization tails (ACT stays pure-exp: the Copy
                        # func lives in every table set, so no table swap):
                        # ACT drains the denominator rows, DVE computes the
                        # approx reciprocals (~51 ULP), GpSimd broadcasts,
                        # DVE multiplies straight out of PSUM into fp16
                        dens, recs, rbs = [], [], []
                        for i in range(2):
                            den = ypool.tile([1, 1024], F32, tag="den",
                                             name=f"den{i}")
                            nc.scalar.activation(den[:], yps[i][64:65, :],
                                                 AF.Copy)
                            dens.append(den)
                        for i in range(2):
                            rec = ypool.tile([1, 1024], F32, tag="rec",
                                             name=f"rec{i}")
                            nc.vector.reciprocal_approx_fast(
                                out=rec[:], in_=dens[i][:])
                            recs.append(rec)
                        for i in range(2):
                            rb = ypool.tile([64, 1024], F32, tag="rb",
                                            name=f"rb{i}")
                            nc.gpsimd.partition_broadcast(
                                rb[:], recs[i][:], channels=64)
                            rbs.append(rb)
                        for i in range(2):
                            nc.vector.tensor_tensor(
                                ynorm[pt][64 * i:64 * i + 64, q0:q0 + 1024],
                                yps[i][0:64, :], rbs[i][:], OP.mult)

                # cover the last block's normalization tail, then the final
                # serial out-projection for qcp=1's tokens
                warm_run(20, "warmtail")
                for j, (tt, oc) in enumerate(
                        (t, o) for t in range(8, 16) for o in range(2)):
                    emit_outproj_group(j, tt, oc,
                                       "act" if j % 2 == 0 else "dve")

    nc.compile()
    return nc


_NC = None


def _get_module():
    global _NC
    if _NC is None:
        _NC = _build_module()
    return _NC


def _host_constants():
    pswap = np.zeros((128, 128), np.float32)
    idx = np.arange(128)
    pswap[idx ^ 1, idx] = 1.0
    return pswap


def _bf(x):
    return np.ascontiguousarray(np.asarray(x, np.float16))


def _prep_in_maps(q, freqs_cis, wq_w, wq_b, wk_w, wk_b, wv_w, wv_b, wo_w, wo_b):
    # F0/F1 [128, S] (identical layout for every head pair on 128 partitions)
    i_of_p = (np.arange(128) % HD) // 2
    sign = np.where(np.arange(128) % 2 == 0, -1.0, 1.0).astype(np.float32)
    f0 = freqs_cis[:, i_of_p, 0].T.copy()                 # [128, S]
    f1 = (freqs_cis[:, i_of_p, 1].T * sign[:, None]).copy()
    pswap = _host_constants()
    ones4 = np.ones((128, 4), ml_dtypes.bfloat16)

    f0 = _bf(f0)
    f1 = _bf(f1)
    pswap = _bf(pswap)

    in_maps = []
    for c in range(NCORES):
        b, hg = c // 4, c % 4
        sl = slice(hg * DL, (hg + 1) * DL)
        in_maps.append({
            "xt": _bf(q[b].T),
            "wqt": _bf(wq_w[sl].T),
            "wkt": _bf(wk_w[sl].T),
            "wvt": _bf(wv_w[sl].T),
            "wot": _bf(wo_w[:, sl].T),
            "qb2": np.ascontiguousarray(wq_b[sl].reshape(2, 128).T),
            "kb2": np.ascontiguousarray(wk_b[sl].reshape(2, 128).T),
            "f0": f0,
            "f1": f1,
            "pswap": pswap,
            "ones4": ones4,
        })
    return in_maps


def kernel(q, freqs_cis, wq_w, wq_b, wk_w, wk_b, wv_w, wv_b, wo_w, wo_b):
    q = np.asarray(q, np.float32)
    freqs_cis = np.asarray(freqs_cis, np.float32)
    wq_w = np.asarray(wq_w, np.float32)
    wq_b = np.asarray(wq_b, np.float32)
    wk_w = np.asarray(wk_w, np.float32)
    wk_b = np.asarray(wk_b, np.float32)
    wv_w = np.asarray(wv_w, np.float32)
    wv_b = np.asarray(wv_b, np.float32)
    wo_w = np.asarray(wo_w, np.float32)
    wo_b = np.asarray(wo_b, np.float32)

    nc = _get_module()
    in_maps = _prep_in_maps(q, freqs_cis, wq_w, wq_b, wk_w, wk_b,
                            wv_w, wv_b, wo_w, wo_b)
    res = run_bass_kernel_spmd(
        nc, in_maps, core_ids=list(range(NCORES)), trace=TRACE)
    LAST_RESULTS[0] = res

    const = (wo_w @ wv_b + wo_b).astype(np.float32)  # V-bias folded through softmax
    out = np.zeros((B, S, D), np.float32)
    for c in range(NCORES):
        out[c // 4] += np.asarray(res.results[c]["partial"], np.float32)
    out += const[None, None, :]
    return out


# revision 12
# speedup vs baseline: 1.0562x; 1.0562x over previous
"""Multi-head attention (QKV proj + RoPE + SDPA + out proj) on 8 TRN2 NeuronCores.

Sharding: batch x head-group. Core c handles batch c//4 and heads
4*(c%4) .. 4*(c%4)+3 (4 of 16 heads, 256 of 1024 feature dims).

Per-core kernel. All matmul operands are bf16 (fp32 PSUM accumulation):
bf16 streams 1 row/cycle at the warm 2.4 GHz PE clock and enables the
compiler's fast-weight-load path (disabled for fp32), so LDWEIGHTS hides
behind the matmul stream. Inputs arrive from HBM already in bf16 (half
the DMA bytes of the fp32 original).

  - QKV projections from host-transposed xT [1024, 2048]:
      Q,K feature-major [dims, tokens] (for scores contraction over head_dim)
      V token-major [tokens, dims] (for attn@V contraction over keys)
  - RoPE on Q/K in feature-major form: q_rot = F0*q + F1*(Pswap@q), with
    F0/F1 [128, S] precomputed on host from freqs_cis and Pswap a constant
    pair-swap permutation matmul.
  - scores computed TRANSPOSED: s[k, q] = sum_d K[d,k] Q[d,q]  (so that
    attn@V needs no transpose); exp via ACT directly from 2-bank PSUM
    groups [128, 1024] with the 1/sqrt(64) scale folded in; no max
    subtraction (|scores/8| <~ 12, safe in f32).
  - attn@V with a ones-row appended per head (M=65): row 64 accumulates
    the softmax denominator for free.
  - normalize straight out of PSUM: ACT reciprocal of the denominator row
    + rank-1 ones x recip broadcast matmul + one DVE multiply into bf16.
  - output projection row-parallel: each core emits a [2048, 1024] bf16
    partial; host sums partials per batch and adds wo_b + wo_w @ wv_b
    (the V bias commutes through softmax-weighted sum: sum_k attn = 1).

Host gather: out[b] = sum_{hg} partial[4*b+hg] + wo_b + wo_w @ wv_b.
"""

import numpy as np
import ml_dtypes

import concourse.bass as bass
import concourse.mybir as mybir
import concourse.tile as tile
from concourse import bacc
import concourse.bass_utils as _bu
from concourse.bass_utils import run_bass_kernel_spmd

F32 = mybir.dt.float32
F32R = mybir.dt.float32r
F16 = mybir.dt.float16
BF16 = mybir.dt.bfloat16
I16 = mybir.dt.int16
PEXP_A = 23.083120654223414   # 128*log2(e)/8
PEXP_B = 16250.8875           # 128*127 - 5.1125 (truncation-tuned magic)
AF = mybir.ActivationFunctionType
OP = mybir.AluOpType

B, S, D = 2, 2048, 1024
NH, HD = 16, 64
NCORES = 8
HPC = 4          # heads per core
DL = HPC * HD    # 256 local dims per core

# set by test harness to request an NTFF trace
TRACE = False
USE_WARM = True
LAST_RESULTS = [None]


def _build_module():
    nc = bacc.Bacc("TRN2", target_bir_lowering=False, debug=False)

    xt_d = nc.dram_tensor("xt", [D, S], F16, kind="ExternalInput")
    wqt_d = nc.dram_tensor("wqt", [D, DL], F16, kind="ExternalInput")
    wkt_d = nc.dram_tensor("wkt", [D, DL], F16, kind="ExternalInput")
    wvt_d = nc.dram_tensor("wvt", [D, DL], F16, kind="ExternalInput")
    wot_d = nc.dram_tensor("wot", [DL, D], F16, kind="ExternalInput")
    qb_d = nc.dram_tensor("qb2", [128, 2], F32, kind="ExternalInput")
    kb_d = nc.dram_tensor("kb2", [128, 2], F32, kind="ExternalInput")
    f0_d = nc.dram_tensor("f0", [128, S], F16, kind="ExternalInput")
    f1_d = nc.dram_tensor("f1", [128, S], F16, kind="ExternalInput")
    psw_d = nc.dram_tensor("pswap", [128, 128], F16, kind="ExternalInput")
    o4_d = nc.dram_tensor("ones4", [128, 4], BF16, kind="ExternalInput")
    out_d = nc.dram_tensor("partial", [S, D], F16, kind="ExternalOutput")

    with tile.TileContext(nc) as tc:
        with (
            tc.tile_pool(name="wts", bufs=1) as wpool,
            tc.tile_pool(name="persist", bufs=1) as ppool,
        ):
            # ---- weights / constants (resident) ----
            wqt = wpool.tile([128, 8, DL], F16, tag="wqt")
            nc.sync.dma_start(
                out=wqt[:], in_=wqt_d.ap().rearrange("(dc p) m -> p dc m", p=128))
            xt_re0 = xt_d.ap().rearrange("(dc p) t -> p dc t", p=128)
            xt0_sb = wpool.tile([128, 8, 512], F16, tag="xt0")
            for dc in range(8):
                nc.sync.dma_start(out=xt0_sb[:, dc, :],
                                  in_=xt_re0[:, dc, 0:512])

            wkt = wpool.tile([128, 8, DL], F16, tag="wkt")
            nc.sync.dma_start(
                out=wkt[:], in_=wkt_d.ap().rearrange("(dc p) m -> p dc m", p=128))
            wvt = wpool.tile([128, 8, DL], F16, tag="wvt")
            nc.sync.dma_start(
                out=wvt[:], in_=wvt_d.ap().rearrange("(dc p) m -> p dc m", p=128))
            qb = wpool.tile([128, 2], F32, tag="qb")
            nc.sync.dma_start(out=qb[:], in_=qb_d.ap())
            kb = wpool.tile([128, 2], F32, tag="kb")
            nc.sync.dma_start(out=kb[:], in_=kb_d.ap())
            f0 = wpool.tile([128, S], F16, tag="f0")
            nc.sync.dma_start(out=f0[:], in_=f0_d.ap())
            f1 = wpool.tile([128, S], F16, tag="f1")
            nc.sync.dma_start(out=f1[:], in_=f1_d.ap())
            psw = wpool.tile([128, 128], F16, tag="pswap")
            nc.sync.dma_start(out=psw[:], in_=psw_d.ap())
            o4 = wpool.tile([128, 4], BF16, tag="o4")
            nc.sync.dma_start(out=o4[:], in_=o4_d.ap())
            wot = wpool.tile([128, 2, D], F16, tag="wot")
            nc.sync.dma_start(
                out=wot[:], in_=wot_d.ap().rearrange("(pt p) o -> p pt o", p=128))

            # ---- persistent activations ----
            qrot = [ppool.tile([128, S], F16, tag=f"qrot{pt}", name=f"qrot{pt}") for pt in range(2)]
            krot = [ppool.tile([128, S], F16, tag=f"krot{pt}", name=f"krot{pt}") for pt in range(2)]
            ynorm = [ppool.tile([128, S], F16, tag=f"ynorm{pt}", name=f"ynorm{pt}") for pt in range(2)]
            vsb = [ppool.tile([128, 260], BF16, tag=f"v{kt}", name=f"vsb{kt}") for kt in range(16)]

            # preload the ACT exp table set and the GpSimd broadcast
            # library during the DMA lead-in (the first partition_broadcast
            # otherwise pays a ~7us library load at a block tail)
            warmact = wpool.tile([1, 1], F32, tag="warmact")
            nc.vector.memset(warmact[:], 0.0)
            nc.scalar.activation(warmact[:], warmact[:], AF.Exp, scale=1.0)
            gwi = wpool.tile([1, 8], F32, tag="gwi")
            nc.vector.memset(gwi[:], 1.0)
            gwo = wpool.tile([8, 8], F32, tag="gwo")
            nc.gpsimd.partition_broadcast(gwo[:], gwi[:], channels=8)

            xt_re = xt_d.ap().rearrange("(dc p) t -> p dc t", p=128)

            # ---- phase 1: QKV projections + RoPE ----
            with (
                tc.tile_pool(name="xt", bufs=2) as xpool,
                tc.tile_pool(name="ptmp", bufs=3) as tpool,
                tc.tile_pool(name="ps2", bufs=2, space="PSUM") as ps2,
            ):
                for qc in range(4):
                    tsl = slice(qc * 512, (qc + 1) * 512)
                    if qc == 0:
                        xt_sb = xt0_sb
                    else:
                        xt_sb = xpool.tile([128, 8, 512], F16, tag="xt")
                        nc.sync.dma_start(out=xt_sb[:], in_=xt_re[:, :, tsl])

                    for wt, bvec, rot in ((wqt, qb, qrot), (wkt, kb, krot)):
                        for pt in range(2):
                            qp = ps2.tile([128, 512], F32, tag="proj")
                            for dc in range(8):
                                nc.tensor.matmul(
                                    qp[:],
                                    wt[:, dc, pt * 128:(pt + 1) * 128],
                                    xt_sb[:, dc, :],
                                    start=(dc == 0), stop=(dc == 7))
                            qsb = tpool.tile([128, 512], F16, tag="qsb")
                            nc.scalar.activation(
                                qsb[:], qp[:], AF.Identity,
                                bias=bvec[:, pt:pt + 1], scale=1.0)
                            sw = ps2.tile([128, 512], F32, tag="swap")
                            nc.tensor.matmul(
                                sw[:], psw[:], qsb[:], start=True, stop=True)
                            t0 = tpool.tile([128, 512], F16, tag="t0")
                            nc.vector.tensor_tensor(
                                t0[:], qsb[:], f0[:, tsl], OP.mult)
                            t1 = tpool.tile([128, 512], F16, tag="t1")
                            nc.vector.tensor_tensor(
                                t1[:], sw[:], f1[:, tsl], OP.mult)
                            nc.vector.tensor_tensor(
                                rot[pt][:, tsl], t0[:], t1[:], OP.add)

                    for tt in range(4):
                        kt = qc * 4 + tt
                        vp = ps2.tile([128, 256], F32, tag="vps")
                        for dc in range(8):
                            nc.tensor.matmul(
                                vp[:],
                                xt_sb[:, dc, tt * 128:(tt + 1) * 128],
                                wvt[:, dc, :],
                                start=(dc == 0), stop=(dc == 7))
                        # single strided ACT copy: vp [128,(4h 64d)] ->
                        # vsb [128, 4h x 65] (leaves the ones column alone)
                        nc.scalar.activation(
                            vsb[kt][:, 0:260].rearrange(
                                "p (h x) -> p h x", x=65)[:, :, 0:64],
                            vp[:].rearrange("p (h d) -> p h d", d=64),
                            AF.Copy)
                        nc.vector.tensor_copy(vsb[kt][:, 64:260:65], o4[:])

            # ---- phase 2: attention ----
            # Heads in pairs: the two 64-row score matmuls use disjoint PE
            # row groups (base partitions 0/64) and run concurrently.
            # "Warm" matmuls are dependency-free full-array fillers written
            # into psum that is about to be overwritten anyway (start=True
            # clears it); they keep the HAM activity monitor at K=8/8
            # through pipeline-fill bubbles.
            #
            # The out-projection for qcp=0 is WOVEN into the (qcp=1, hp=0)
            # attention block: its matmuls fill the PE gaps of the ACT
            # (exp)-paced score/attnV pipeline, and its PSUM->SBUF drains go
            # to the otherwise-idle DVE. Only qcp=1's out-projection runs
            # serially at the end (with ACT/DVE alternating drains).
            with (
                tc.tile_pool(name="exp", bufs=6) as epool,
                tc.tile_pool(name="ysb", bufs=2) as ypool,
                tc.tile_pool(name="opool", bufs=4) as opool,
                tc.tile_pool(name="ps3s", bufs=1, space="PSUM") as ps3s,
                tc.tile_pool(name="ps3y", bufs=1, space="PSUM") as ps3y,
            ):
                def warm_run(n, name):
                    if not USE_WARM:
                        return
                    wt_ = ps3s.tile([128, 512], F32, tag="s0", name=name)
                    for _ in range(n):
                        nc.tensor.matmul(wt_[:], psw[:], wot[:, 0, 0:512],
                                         start=True, stop=True)

                def emit_outproj_group(j, tt, oc, drain):
                    op = ps3s.tile([128, 512], F32, tag=("s0" if j % 2 == 0 else "s1a"),
                                   name=f"op{tt}_{oc}")
                    for pt2 in range(2):
                        nc.tensor.matmul(
                            op[:],
                            ynorm[pt2][:, tt * 128:(tt + 1) * 128],
                            wot[:, pt2, oc * 512:(oc + 1) * 512],
                            start=(pt2 == 0), stop=(pt2 == 1))
                    osb = opool.tile([128, 512], F16, tag="osb",
                                     name=f"osb{tt}_{oc}")
                    if drain == "dve":
                        nc.vector.tensor_copy(osb[:], op[:])
                    else:
                        nc.scalar.activation(osb[:], op[:], AF.Copy)
                    nc.sync.dma_start(
                        out=out_d.ap()[tt * 128:(tt + 1) * 128,
                                       oc * 512:(oc + 1) * 512],
                        in_=osb[:])

                for qcp in range(2):
                    q0 = qcp * 1024
                    for hp in range(2):
                        pt = hp
                        if (qcp, hp) != (0, 0):
                            warm_run(4, f"warmhp{qcp}_{hp}")
                        yps = [ps3y.tile([65, 1024], F32, tag=f"y{i}",
                                         name=f"yp{i}") for i in range(2)]
                        # out-proj groups of the previous qcp to weave into
                        # this block's PE gaps (2 per kt over kt=2..9)
                        woven = ([(j, t, o) for j, (t, o) in enumerate(
                                     (t, o) for t in range(8) for o in range(2))]
                                 if (qcp == 1 and hp == 0) else [])

                        def emit_scores_exp(kt, nwarm):
                            exs = []
                            # head i=0: 2-bank psum tile, full ACT exp
                            sp = ps3s.tile([128, 1024], F32, tag="s0",
                                           name="sp0")
                            for w in range(nwarm if USE_WARM else 0):
                                nc.tensor.matmul(
                                    sp[:, 0:512], psw[:], wot[:, 0, 0:512],
                                    start=True, stop=True)
                            for qh in range(2):
                                nc.tensor.matmul(
                                    sp[:, qh * 512:(qh + 1) * 512],
                                    krot[pt][0:64, kt * 128:(kt + 1) * 128],
                                    qrot[pt][0:64,
                                             q0 + qh * 512:q0 + (qh + 1) * 512],
                                    start=True, stop=True)
                            ex = epool.tile([128, 1024], BF16, tag="e",
                                            name="ex0")
                            nc.scalar.activation(ex[:], sp[:], AF.Exp,
                                                 scale=0.125)
                            exs.append(ex)
                            # head i=1: TWO single-bank psum tiles so the ACT
                            # half and the DVE pseudo-exp half release their
                            # slots independently (keeps the DVE out of the
                            # PE's score-psum rotation chain)
                            spa = ps3s.tile([128, 512], F32, tag="s1a",
                                            name="sp1a")
                            spb = ps3s.tile([128, 512], F32, tag="s1b",
                                            name="sp1b")
                            for qh, spx in ((0, spa), (1, spb)):
                                nc.tensor.matmul(
                                    spx[:],
                                    krot[pt][64:128, kt * 128:(kt + 1) * 128],
                                    qrot[pt][64:128,
                                             q0 + qh * 512:q0 + (qh + 1) * 512],
                                    start=True, stop=True)
                            ex1 = epool.tile([128, 1024], BF16, tag="e",
                                             name="ex1")
                            nc.scalar.activation(ex1[:, 0:512], spa[:],
                                                 AF.Exp, scale=0.125)
                            # exp bit trick: bf16(2^y) bits = trunc(s*A+B)
                            # (~3.3% max weight err; numerator and
                            # denominator share it, so much cancels)
                            nc.vector.tensor_scalar(
                                out=ex1[:, 512:1024].bitcast(I16),
                                in0=spb[:],
                                scalar1=PEXP_A, scalar2=PEXP_B,
                                op0=OP.mult, op1=OP.add)
                            exs.append(ex1)
                            return exs

                        def emit_attnv(kt, exs):
                            for i in range(2):
                                h = 2 * hp + i
                                for qh in range(2):
                                    nc.tensor.matmul(
                                        yps[i][:, qh * 512:(qh + 1) * 512],
                                        vsb[kt][:, 65 * h:65 * h + 65],
                                        exs[i][:, qh * 512:(qh + 1) * 512],
                                        start=(kt == 0), stop=(kt == 15))

                        # software pipeline: attnV trails scores/exp by two
                        # iterations so exp never queues behind attnV and the
                        # previous block's normalization tail (which gates
                        # attnv(0) via the yps slot) stays off the PE path
                        exq = [emit_scores_exp(0, 2), emit_scores_exp(1, 1)]
                        for kt in range(2, 16):
                            exq.append(emit_scores_exp(kt, 1 if kt < 3 else 0))
                            emit_attnv(kt - 2, exq[kt - 2])
                            if woven and 2 <= kt <= 9:
                                for j, t, o in woven[(kt - 2) * 2:(kt - 1) * 2]:
                                    emit_outproj_group(j, t, o, "dve")
                        emit_attnv(14, exq[14])
                        emit_attnv(15, exq[15])

                        # normalization tails (ACT stays pure-exp: the Copy
                        # func lives in every table set, so no table swap):
                        # ACT drains the denominator rows, DVE computes the
                        # approx reciprocals (~51 ULP), GpSimd broadcasts,
                        # DVE multiplies straight out of PSUM into fp16
                        dens, recs, rbs = [], [], []
                        for i in range(2):
                            den = ypool.tile([1, 1024], F32, tag="den",
                                             name=f"den{i}")
                            nc.scalar.activation(den[:], yps[i][64:65, :],
                                                 AF.Copy)
                            dens.append(den)
                        for i in range(2):
                            rec = ypool.tile([1, 1024], F32, tag="rec",
                                             name=f"rec{i}")
                            nc.vector.reciprocal_approx_fast(
                                out=rec[:], in_=dens[i][:])
                            recs.append(rec)
                        for i in range(2):
                            rb = ypool.tile([64, 1024], F32, tag="rb",
                                            name=f"rb{i}")
                            nc.gpsimd.partition_broadcast(
                                rb[:], recs[i][:], channels=64)
                            rbs.append(rb)
                        for i in range(2):
                            nc.vector.tensor_tensor(
                                ynorm[pt][64 * i:64 * i + 64, q0:q0 + 1024],
                                yps[i][0:64, :], rbs[i][:], OP.mult)

                # cover the last block's normalization tail, then the final
                # serial out-projection for qcp=1's tokens
                warm_run(20, "warmtail")
                for j, (tt, oc) in enumerate(
                        (t, o) for t in range(8, 16) for o in range(2)):
                    emit_outproj_group(j, tt, oc,
                                       "act" if j % 2 == 0 else "dve")

    nc.compile()
    return nc


_NC = None


def _get_module():
    global _NC
    if _NC is None:
        _NC = _build_module()
    return _NC


def _host_constants():
    pswap = np.zeros((128, 128), np.float32)
    idx = np.arange(128)
    pswap[idx ^ 1, idx] = 1.0
    return pswap


def _bf(x):
    return np.ascontiguousarray(np.asarray(x, np.float16))


def _prep_in_maps(q, freqs_cis, wq_w, wq_b, wk_w, wk_b, wv_w, wv_b, wo_w, wo_b):
    # F0/F1 [128, S] (identical layout for every head pair on 128 partitions)
    i_of_p = (np.arange(128) % HD) // 2
    sign = np.where(np.arange(128) % 2 == 0, -1.0, 1.0).astype(np.float32)
    f0 = freqs_cis[:, i_of_p, 0].T.copy()                 # [128, S]
    f1 = (freqs_cis[:, i_of_p, 1].T * sign[:, None]).copy()
    pswap = _host_constants()
    ones4 = np.ones((128, 4), ml_dtypes.bfloat16)

    f0 = _bf(f0)
    f1 = _bf(f1)
    pswap = _bf(pswap)

    in_maps = []
    for c in range(NCORES):
        b, hg = c // 4, c % 4
        sl = slice(hg * DL, (hg + 1) * DL)
        in_maps.append({
            "xt": _bf(q[b].T),
            "wqt": _bf(wq_w[sl].T),
            "wkt": _bf(wk_w[sl].T),
            "wvt": _bf(wv_w[sl].T),
            "wot": _bf(wo_w[:, sl].T),
            "qb2": np.ascontiguousarray(wq_b[sl].reshape(2, 128).T),
            "kb2": np.ascontiguousarray(wk_b[sl].reshape(2, 128).T),
            "f0": f0,
            "f1": f1,
            "pswap": pswap,
            "ones4": ones4,
        })
    return in_maps


def kernel(q, freqs_cis, wq_w, wq_b, wk_w, wk_b, wv_w, wv_b, wo_w, wo_b):
    q = np.asarray(q, np.float32)
    freqs_cis = np.asarray(freqs_cis, np.float32)
    wq_w = np.asarray(wq_w, np.float32)
    wq_b = np.asarray(wq_b, np.float32)
    wk_w = np.asarray(wk_w, np.float32)
    wk_b = np.asarray(wk_b, np.float32)
    wv_w = np.asarray(wv_w, np.float32)
    wv_b = np.asarray(wv_b, np.float32)
    wo_w = np.asarray(wo_w, np.float32)
    wo_b = np.asarray(wo_b, np.float32)

    nc = _get_module()
    in_maps = _prep_in_maps(q, freqs_cis, wq_w, wq_b, wk_w, wk_b,
                            wv_w, wv_b, wo_w, wo_b)
    res = run_bass_kernel_spmd(
        nc, in_maps, core_ids=list(range(NCORES)), trace=TRACE)
    LAST_RESULTS[0] = res

    const = (wo_w @ wv_b + wo_b).astype(np.float32)  # V-bias folded through softmax
    out = np.zeros((B, S, D), np.float32)
    for c in range(NCORES):
        out[c // 4] += np.asarray(res.results[c]["partial"], np.float32)
    out += const[None, None, :]
    return out
